# revision 1
# baseline (speedup 1.0000x reference)
"""Trainium2 Bass kernel for nn_Cross_Attention_18425409700231.

Per-sample channel attention (16 heads x 8 channels, L2-normalized over
spatial, softmax over in-head channels) followed by a conv block
(3x3 conv -> LeakyReLU -> 1x1 conv -> reflect-pad depthwise 3x3 ->
LeakyReLU, plus 1x1 shortcut) and a residual add.

Sharding: data-parallel over batch B=8 -> one sample per NeuronCore.

Device algorithm per core (sample b), all layouts [C=128 partitions, H*W]:
  A. Gram matrix G = x1 @ x1^T (contract over 16384 spatial) via
     PE-transposed bf16 chunks; norms from diag(G); S = rn*G*rn (one PE
     transpose for the column scale, exploiting symmetry); E = exp(S*temp)
     masked to the 16 block-diagonal 8x8 head blocks.
  B. Attention apply: P = E @ x2 (f32r matmuls), row-scaled by 1/rowsum(E)
     during the PSUM->SBUF copy, written into a zero-padded 130x130 buffer.
  C. conv1 3x3 as 9 accumulated matmuls per 4-row band from the padded
     buffer; bias+LeakyReLU fused in the PSUM->SBUF copy into a second
     (reflect-)padded buffer.
  D. conv2(1x1) and depthwise 3x3 fused into 9 taps of modified weights
     W2t[t] = dw_w[:,t] * conv2_w (reflect pad commutes with 1x1 conv);
     bias'+LeakyReLU; shortcut 1x1 matmul on the attention output; final
     out = lrelu(...) + (sc + sc_b) + x1, streamed back to DRAM.
  B/C/D are emitted interleaved per 4-row band (with dependency lags) so
  the PE never drains while DMA streams x2/x1 in and the result out.
"""

import numpy as np
import ml_dtypes

B, C, H, W = 8, 128, 128, 128
HW = H * W
HEADS, HEAD_C = 16, 8
SLOPE = 0.2
EPS = 1e-12
PW = W + 2  # padded width
NB = H // 4  # 32 bands of 4 rows

_cache = {}


def _build_program(debug=False):
    import concourse.bass as bass
    import concourse.tile as tile
    import concourse.mybir as mybir
    from concourse import bacc

    dt = mybir.dt
    f32, f32r, bf16 = dt.float32, dt.float32r, dt.bfloat16
    u32 = dt.uint32
    AF = mybir.ActivationFunctionType
    ALU = mybir.AluOpType
    AX = mybir.AxisListType

    nc = bacc.Bacc("TRN2", num_devices=8)

    x1 = nc.dram_tensor("x1", [C, HW], f32, kind="ExternalInput").ap()
    x1h = nc.dram_tensor("x1h", [C, HW], bf16, kind="ExternalInput").ap()
    x2 = nc.dram_tensor("x2", [C, HW], f32r, kind="ExternalInput").ap()
    wc1 = nc.dram_tensor("wc1", [C, 9, C], f32r, kind="ExternalInput").ap()
    wc2 = nc.dram_tensor("wc2", [C, 9, C], f32r, kind="ExternalInput").ap()
    wsc = nc.dram_tensor("wsc", [C, C], f32r, kind="ExternalInput").ap()
    scl = nc.dram_tensor("scl", [C, 4], f32, kind="ExternalInput").ap()
    bmask = nc.dram_tensor("bmask", [C, C], f32, kind="ExternalInput").ap()
    idf = nc.dram_tensor("idf", [C, C], f32, kind="ExternalInput").ap()
    idb = nc.dram_tensor("idb", [C, C], bf16, kind="ExternalInput").ap()
    out = nc.dram_tensor("out", [C, HW], f32, kind="ExternalOutput").ap()
    if debug:
        dbg_g = nc.dram_tensor("dbg_g", [C, C], f32, kind="ExternalOutput").ap()
        dbg_em = nc.dram_tensor("dbg_em", [C, C], f32, kind="ExternalOutput").ap()
        dbg_rinv = nc.dram_tensor("dbg_rinv", [C, 1], f32, kind="ExternalOutput").ap()
        dbg_ph = nc.dram_tensor("dbg_ph", [C, (H + 2) * PW], f32, kind="ExternalOutput").ap()

    taps = [(dy, dx) for dy in range(3) for dx in range(3)]

    with tile.TileContext(nc) as tc:
        with (
            tc.tile_pool(name="consts", bufs=1) as consts,
            tc.tile_pool(name="pads", bufs=1) as pads,
            tc.tile_pool(name="attn", bufs=1) as attn,
            tc.tile_pool(name="streams", bufs=2) as streams,
            tc.tile_pool(name="bands", bufs=3) as bands,
        ):
            # ---- constants to SBUF ----
            idbs = consts.tile([C, C], bf16)
            nc.sync.dma_start(out=idbs, in_=idb)
            w1s = consts.tile([C, 9, C], f32r)
            nc.gpsimd.dma_start(out=w1s, in_=wc1)
            w2s = consts.tile([C, 9, C], f32r)
            nc.gpsimd.dma_start(out=w2s, in_=wc2)
            wscs = consts.tile([C, C], f32r)
            nc.gpsimd.dma_start(out=wscs, in_=wsc)
            scls = consts.tile([C, 4], f32)
            nc.gpsimd.dma_start(out=scls, in_=scl)
            bmasks = consts.tile([C, C], f32)
            nc.gpsimd.dma_start(out=bmasks, in_=bmask)
            idfs = consts.tile([C, C], f32)
            nc.gpsimd.dma_start(out=idfs, in_=idf)
            b1_ap = scls[:, 0:1]
            b2_ap = scls[:, 1:2]
            bsc_ap = scls[:, 2:3]
            temp_ap = scls[:, 3:4]

            # ---- persistent padded buffers ----
            p2x = pads.tile([C, H + 2, PW], f32r)  # x2, zero-pad
            ph = pads.tile([C, H + 2, PW], f32r)   # conv1 out, reflect-pad

            # zero borders of p2x; interior streamed straight from DRAM
            nc.vector.memset(p2x[:, 0:1, :].bitcast(u32), 0)
            nc.vector.memset(p2x[:, H + 1 : H + 2, :].bitcast(u32), 0)
            nc.vector.memset(p2x[:, 1 : H + 1, 0:1].bitcast(u32), 0)
            nc.vector.memset(p2x[:, 1 : H + 1, PW - 1 : PW].bitcast(u32), 0)

            # ================= phase A: Gram + softmax =================
            with (
                tc.tile_pool(name="psG", bufs=1, space="PSUM") as psG,
                tc.tile_pool(name="psT", bufs=3, space="PSUM") as psT,
            ):
                gps = psG.tile([C, C], f32)
                kk = 0
                col0 = 0
                for ncols in (1024, 3072, 4096, 4096, 4096):
                    x1bt = streams.tile(
                        [C, ncols], bf16, bufs=3, tag="x1bt", name="x1bt"
                    )
                    nc.sync.dma_start(
                        out=x1bt, in_=x1h[:, col0 : col0 + ncols]
                    )
                    col0 += ncols
                    for g in range(ncols // 512):  # batches of 4 chunks of 128
                        tp = psT.tile([C, 4, C], bf16)
                        for i in range(4):
                            k = g * 4 + i
                            nc.tensor.transpose(
                                out=tp[:, i, :],
                                in_=x1bt[:, k * 128 : (k + 1) * 128],
                                identity=idbs,
                            )
                        tsb = streams.tile([C, 4, C], bf16, bufs=4)
                        nc.vector.tensor_copy(out=tsb, in_=tp)
                        for i in range(4):
                            nc.tensor.matmul(
                                out=gps,
                                lhsT=tsb[:, i, :],
                                rhs=tsb[:, i, :],
                                start=(kk == 0),
                                stop=(kk == 127),
                                skip_group_check=True,
                            )
                            kk += 1

                # stream x2 into the padded buffer (no deps; overlaps A tail)
                for j in range(8):
                    nc.sync.dma_start(
                        out=p2x[:, 1 + 16 * j : 17 + 16 * j, 1 : 1 + C],
                        in_=x2[:, j * 2048 : (j + 1) * 2048],
                    )

                # diag -> norms -> rn
                gi = attn.tile([C, C], f32)
                nc.vector.tensor_mul(out=gi, in0=gps, in1=idfs)
                diag = attn.tile([C, 1], f32)
                nc.vector.reduce_sum(out=diag, in_=gi, axis=AX.X)
                norm = attn.tile([C, 1], f32)
                nc.scalar.activation(out=norm, in_=diag, func=AF.Sqrt)
                nc.vector.tensor_scalar_max(out=norm, in0=norm, scalar1=EPS)
                rn = attn.tile([C, 1], f32)
                nc.vector.reciprocal(out=rn, in_=norm)

                # S = diag(rn) G diag(rn) via row-scale, transpose, row-scale
                s1 = attn.tile([C, C], f32)
                nc.vector.tensor_scalar_mul(out=s1, in0=gps, scalar1=rn)
                with tc.tile_pool(name="psS", bufs=1, space="PSUM") as psS:
                    s1t = psS.tile([C, C], f32)
                    nc.tensor.transpose(out=s1t, in_=s1, identity=idfs)
                    s2 = attn.tile([C, C], f32)
                    nc.vector.tensor_scalar_mul(out=s2, in0=s1t, scalar1=rn)

                # E = exp(S * temp) * blockmask ; rinv = 1/rowsum(E)
                e0 = attn.tile([C, C], f32)
                nc.scalar.activation(out=e0, in_=s2, func=AF.Exp, scale=temp_ap)
                em = attn.tile([C, C], f32r)
                nc.vector.tensor_mul(out=em, in0=e0, in1=bmasks)
                rs = attn.tile([C, 1], f32)
                nc.vector.reduce_sum(out=rs, in_=em, axis=AX.X)
                rinv = attn.tile([C, 1], f32)
                nc.vector.reciprocal(out=rinv, in_=rs)

                # fused attention+conv weights: L_t = E diag(rinv) w1s_t,
                # Lsc = E diag(rinv) wsc  (E symmetric), so that
                # conv1(P) = sum_t L_t^T @ x2_shift with P never materialized.
                ltp = []
                with tc.tile_pool(name="psW", bufs=2, space="PSUM") as psW:
                    for p in range(5):  # pairs of taps -> N=256 matmuls
                        rt = attn.tile([C, 2, C], f32r, name=f"rt{p}", tag="rt")
                        if p < 4:
                            nc.vector.tensor_scalar_mul(
                                out=rt, in0=w1s[:, 2 * p : 2 * p + 2, :], scalar1=rinv
                            )
                        else:
                            nc.vector.tensor_scalar_mul(
                                out=rt[:, 0, :], in0=w1s[:, 8, :], scalar1=rinv
                            )
                            nc.vector.tensor_scalar_mul(
                                out=rt[:, 1, :], in0=wscs, scalar1=rinv
                            )
                        lps = psW.tile([C, 2, C], f32, name=f"lps{p}", tag="lps")
                        nc.tensor.matmul(
                            out=lps, lhsT=em, rhs=rt, start=True, stop=True
                        )
                        lt = attn.tile([C, 2, C], f32r, name=f"lt{p}")
                        nc.scalar.activation(out=lt, in_=lps, func=AF.Copy)
                        ltp.append(lt)
                lts = [ltp[t // 2][:, t % 2, :] for t in range(10)]
                if debug:
                    gsb = attn.tile([C, C], f32)
                    nc.vector.tensor_copy(out=gsb, in_=gps)
                    nc.sync.dma_start(out=dbg_g, in_=gsb)
                    nc.gpsimd.dma_start(out=dbg_em, in_=em)
                    nc.sync.dma_start(out=dbg_rinv, in_=rinv)

            # ============ phases C/D interleaved per 4-row band ============
            # C band b reads p2x rows 4b-1..4b+4 (streamed-in x2)
            # D band b reads ph  rows 4b-1..4b+4  -> needs C bands <= b+1
            with (
                tc.tile_pool(name="psC", bufs=3, space="PSUM") as psC,
                tc.tile_pool(name="psD", bufs=3, space="PSUM") as psD,
                tc.tile_pool(name="psS2", bufs=2, space="PSUM") as psS2,
                tc.tile_pool(name="x1st", bufs=2) as x1st,
            ):
                state = {}

                def emit_C(b):
                    y0 = 4 * b
                    cps = psC.tile([C, 4, C], f32)
                    for t, (ddy, ddx) in enumerate(taps):
                        nc.tensor.matmul(
                            out=cps,
                            lhsT=lts[t],
                            rhs=p2x[:, y0 + ddy : y0 + ddy + 4, ddx : ddx + C],
                            start=(t == 0),
                            stop=(t == 8),
                        )
                    nc.scalar.activation(
                        out=ph[:, 1 + y0 : 5 + y0, 1 : 1 + C],
                        in_=cps,
                        func=AF.Prelu,
                        bias=b1_ap,
                        alpha=SLOPE,
                    )
                    # incremental reflect pad of the left/right columns
                    nc.gpsimd.tensor_copy(
                        out=ph[:, 1 + y0 : 5 + y0, 0:1],
                        in_=ph[:, 1 + y0 : 5 + y0, 2:3],
                    )
                    nc.gpsimd.tensor_copy(
                        out=ph[:, 1 + y0 : 5 + y0, PW - 1 : PW],
                        in_=ph[:, 1 + y0 : 5 + y0, PW - 3 : PW - 2],
                    )
                    if b == 0:
                        # reflect pad top row (incl. corners)
                        nc.gpsimd.tensor_copy(out=ph[:, 0:1, :], in_=ph[:, 2:3, :])
                    if b == NB - 1:
                        nc.gpsimd.tensor_copy(
                            out=ph[:, H + 1 : H + 2, :], in_=ph[:, H - 1 : H, :]
                        )

                def emit_D(b):
                    y0 = 4 * b
                    if b % 2 == 0:
                        x1b2 = x1st.tile([C, 1024], f32)
                        nc.gpsimd.dma_start(
                            out=x1b2, in_=x1[:, y0 * W : (y0 + 8) * W]
                        )
                        otile = x1st.tile([C, 1024], f32, tag="otile")
                        state["x1b2"] = x1b2
                        state["otile"] = otile
                    x1b2 = state["x1b2"]
                    otile = state["otile"]
                    x1b = x1b2[:, (b % 2) * 512 : (b % 2 + 1) * 512].rearrange(
                        "p (a b) -> p a b", a=4
                    )
                    dps = psD.tile([C, 4, C], f32)
                    for t, (ddy, ddx) in enumerate(taps):
                        nc.tensor.matmul(
                            out=dps,
                            lhsT=w2s[:, t, :],
                            rhs=ph[:, y0 + ddy : y0 + ddy + 4, ddx : ddx + C],
                            start=(t == 0),
                            stop=(t == 8),
                        )
                    sps = psS2.tile([C, 4, C], f32)
                    nc.tensor.matmul(
                        out=sps,
                        lhsT=lts[9],
                        rhs=p2x[:, 1 + y0 : 5 + y0, 1 : 1 + C],
                        start=True,
                        stop=True,
                    )
                    h3 = bands.tile([C, 4, C], f32)
                    nc.scalar.activation(
                        out=h3, in_=dps, func=AF.Prelu, bias=b2_ap, alpha=SLOPE
                    )
                    ob = otile[:, (b % 2) * 512 : (b % 2 + 1) * 512].rearrange(
                        "p (a b) -> p a b", a=4
                    )
                    # (sc + bsc) + x1 runs on DVE in parallel with the
                    # Prelu on ACT; h3 joins last.
                    nc.vector.scalar_tensor_tensor(
                        out=ob,
                        in0=sps,
                        scalar=bsc_ap,
                        in1=x1b,
                        op0=ALU.add,
                        op1=ALU.add,
                    )
                    nc.vector.tensor_add(out=ob, in0=ob, in1=h3)
                    if b == NB - 2:
                        # flush first half of the last pair early
                        nc.sync.dma_start(
                            out=out[:, y0 * W : (y0 + 4) * W], in_=otile[:, 0:512]
                        )
                    elif b == NB - 1:
                        nc.sync.dma_start(
                            out=out[:, y0 * W : (y0 + 4) * W], in_=otile[:, 512:1024]
                        )
                    elif b % 2 == 1:
                        nc.sync.dma_start(
                            out=out[:, (y0 - 4) * W : (y0 + 4) * W], in_=otile
                        )

                for k in range(NB + 1):
                    if k < NB:
                        emit_C(k)
                    if k >= 1:
                        emit_D(k - 1)
                        if debug and k == NB:
                            nc.gpsimd.dma_start(out=dbg_ph, in_=ph)

    nc.compile()
    return nc


def _prep_consts(temperature, conv1_w, conv2_w, dw_w, conv1_b, conv2_b, dw_b, sc_b, sc_w):
    f32 = np.float32
    conv1_w = np.asarray(conv1_w, f32)
    conv2_w = np.asarray(conv2_w, f32)
    dw_w = np.asarray(dw_w, f32)
    sc_w = np.asarray(sc_w, f32)
    # conv1 taps as lhsT: wc1[ci, t, co] = conv1_w[co, ci, dy, dx]
    wc1 = np.ascontiguousarray(conv1_w.transpose(1, 2, 3, 0).reshape(C, 9, C))
    # fused conv2+dw taps: wc2[ci, t, co] = conv2_w[co, ci] * dw_w[co, t]
    A2 = conv2_w[:, :, 0, 0]                      # [co, ci]
    Dw = dw_w[:, 0, :, :].reshape(C, 9)           # [co, t]
    wc2 = np.ascontiguousarray(np.einsum("oc,ot->cto", A2, Dw).astype(f32))
    wsc = np.ascontiguousarray(sc_w[:, :, 0, 0].T.astype(f32))
    b2p = np.asarray(dw_b, f32) + np.asarray(conv2_b, f32) * Dw.sum(axis=1)
    temp_b = np.repeat(np.asarray(temperature, f32).reshape(HEADS), HEAD_C)
    scl = np.ascontiguousarray(
        np.stack(
            [np.asarray(conv1_b, f32), b2p, np.asarray(sc_b, f32), temp_b], axis=1
        )
    )  # [128, 4]
    bmask = np.kron(np.eye(HEADS, dtype=f32), np.ones((HEAD_C, HEAD_C), f32))
    idf = np.eye(C, dtype=f32)
    idb = np.eye(C, dtype=ml_dtypes.bfloat16)
    return dict(
        wc1=wc1, wc2=wc2, wsc=wsc, scl=scl,
        bmask=np.ascontiguousarray(bmask),
        idf=np.ascontiguousarray(idf),
        idb=np.ascontiguousarray(idb),
    )


def kernel(
    x1, x2, temperature, conv1_w, conv1_b, conv2_w, conv2_b, dw_w, dw_b, sc_w, sc_b
):
    from concourse.bass_utils import run_bass_kernel_spmd

    if "nc" not in _cache:
        _cache["nc"] = _build_program()
    nc = _cache["nc"]

    x1 = np.ascontiguousarray(np.asarray(x1, np.float32))
    x2 = np.ascontiguousarray(np.asarray(x2, np.float32))
    consts = _prep_consts(
        temperature, conv1_w, conv2_w, dw_w, conv1_b, conv2_b, dw_b, sc_b, sc_w
    )
    in_maps = []
    for b in range(B):
        m = dict(consts)
        m["x1"] = x1[b].reshape(C, HW)
        m["x1h"] = x1[b].reshape(C, HW).astype(ml_dtypes.bfloat16)
        m["x2"] = x2[b].reshape(C, HW)
        in_maps.append(m)

    res = run_bass_kernel_spmd(nc, in_maps, core_ids=list(range(B)))
    outs = [res.results[b]["out"].reshape(C, H, W) for b in range(B)]
    return np.stack(outs, axis=0)



# revision 4
# speedup vs baseline: 2.0976x; 2.0976x over previous
"""Trainium2 Bass kernel for nn_Cross_Attention_18425409700231.

Per-sample channel attention (16 heads x 8 channels, L2-normalized over
spatial, softmax over in-head channels) followed by a conv block
(3x3 conv -> LeakyReLU -> 1x1 conv -> reflect-pad depthwise 3x3 ->
LeakyReLU, plus 1x1 shortcut) and a residual add.

Sharding: data-parallel over batch B=8 -> one sample per NeuronCore.

Device algorithm per core (sample b):
  A. Gram matrix G = x1 @ x1^T from a HOST-pretransposed fp8 copy of x1
     ([spatial, chan] layout, so no PE transposes), accumulated with fp8
     DoubleRow matmuls (two 128-deep k-tiles per instruction, 0.5
     cycles/row).  Norms from diag(G); S = rn*G*rn; E = exp(S*temp)
     masked to the 16 block-diagonal 8x8 head blocks; rinv = 1/rowsum.
  B. Fused attention+conv weights L_t = E diag(rinv) w1_t and
     Lsc = E diag(rinv) wsc, cast to fp8 (x64 scale) and packed in
     DoubleRow tap-pairs; the shortcut/odd-tap zero k-tiles carry fp8
     quantization-error compensation terms instead of zeros.
  C. conv1 3x3 on host-zero-padded fp8 x2: per 8-row band, 10 DoubleRow
     matmuls (2 taps each) accumulating 2 PSUM banks; one ACT pass does
     bias+LeakyReLU and writes the reflect-padded fp8 conv1 buffer.
  D. conv2(1x1)+depthwise 3x3 fused into 9 taps (host-packed fp8
     DoubleRow pairs) + the shortcut pair; bias+LeakyReLU on ACT;
     epilogue (shortcut + sc_b + x1 + h3) on DVE in bf16; out streamed
     to DRAM as bf16 and upcast on host.
  C/D are interleaved per band with a 1-band lag so the PE never drains.
"""

import numpy as np
import ml_dtypes

B, C, H, W = 8, 128, 128, 128
HW = H * W
HEADS, HEAD_C = 16, 8
SLOPE = 0.2
EPS = 1e-12
PW = W + 2          # padded width
ROWS = 8            # band height
NB = H // ROWS      # 16 bands
S_L = 64.0          # fp8 scale for attention-fused conv1/shortcut weights
S_PH = 4.0          # fp8 scale for the conv1 activation buffer
S_2 = 256.0         # fp8 scale for fused conv2*dw weights

# DoubleRow tap pairs: (tap_a, tap_b, window row offset dy, col offset dx,
# elem stride between the two windows).  Taps are dy*3+dx.
PAIRS = [
    (0, 1, 0, 0, 1),      # (0,0)+(0,1): shift right by 1
    (2, 5, 0, 2, PW),     # (0,2)+(1,2): shift down by 1 row
    (3, 4, 1, 0, 1),      # (1,0)+(1,1)
    (6, 7, 2, 0, 1),      # (2,0)+(2,1)
    (8, -1, 2, 2, 0),     # (2,2) + its fp8 compensation (same window)
]

_cache = {}


def _build_program():
    import concourse.bass as bass
    import concourse.tile as tile
    import concourse.mybir as mybir
    from concourse import bacc

    dt = mybir.dt
    f32, f32r, bf16, f8 = dt.float32, dt.float32r, dt.bfloat16, dt.float8e4
    u32 = dt.uint32
    AF = mybir.ActivationFunctionType
    ALU = mybir.AluOpType
    AX = mybir.AxisListType
    PM = mybir.MatmulPerfMode

    nc = bacc.Bacc("TRN2", num_devices=8)

    x1t8 = nc.dram_tensor("x1t8", [C, HW], f8, kind="ExternalInput").ap()
    x2p8 = nc.dram_tensor("x2p8", [C, (H + 2) * PW], f8, kind="ExternalInput").ap()
    x1b16 = nc.dram_tensor("x1b16", [C, HW], bf16, kind="ExternalInput").ap()
    w1p = nc.dram_tensor("w1p", [C, 5, 2, C], f32r, kind="ExternalInput").ap()
    w2p = nc.dram_tensor("w2p", [C, 5, 2, C], f8, kind="ExternalInput").ap()
    sclv = nc.dram_tensor("sclv", [C, 4], f32, kind="ExternalInput").ap()
    bmask = nc.dram_tensor("bmask", [C, C], f32, kind="ExternalInput").ap()
    idf = nc.dram_tensor("idf", [C, C], f32, kind="ExternalInput").ap()
    out = nc.dram_tensor("out", [C, HW], bf16, kind="ExternalOutput").ap()

    def pair_rhs(buf, y, x, strd):
        """[C, 2, ROWS//2*?, C] DoubleRow moving AP: two 3x3-tap windows of a
        padded [C, 130, 130] buffer, 4 rows each, dim1 = the tap pair."""
        base = buf[:, y : y + 4, x : x + C].unsqueeze(1)
        if strd == 0:
            return base.broadcast_to([C, 2, 4, C])
        ap = [list(p) for p in base.ap]
        ap[1] = [strd, 2]
        return bass.AP(tensor=base.tensor, offset=base.offset, ap=ap)

    with tile.TileContext(nc) as tc:
        with (
            tc.tile_pool(name="consts", bufs=1) as consts,
            tc.tile_pool(name="pads", bufs=1) as pads,
            tc.tile_pool(name="attn", bufs=1) as attn,
            tc.tile_pool(name="bands", bufs=2) as bands,
            tc.tile_pool(name="x1st", bufs=2) as x1st,
        ):
            # ---- constants to SBUF (Pool SWDGE queue; off the main stream) ----
            w1s = consts.tile([C, 5, 2, C], f32r)
            nc.gpsimd.dma_start(out=w1s, in_=w1p)
            w2s = consts.tile([C, 5, 2, C], f8)
            nc.gpsimd.dma_start(out=w2s, in_=w2p)
            scls = consts.tile([C, 4], f32)
            nc.gpsimd.dma_start(out=scls, in_=sclv)
            bmasks = consts.tile([C, C], f32)
            nc.gpsimd.dma_start(out=bmasks, in_=bmask)
            idfs = consts.tile([C, C], f32)
            nc.gpsimd.dma_start(out=idfs, in_=idf)
            b1_ap = scls[:, 0:1]    # S_PH * conv1_b
            b2_ap = scls[:, 1:2]    # dw_b + conv2_b * sum(dw)
            bsc_ap = scls[:, 2:3]   # sc_b
            temp_ap = scls[:, 3:4]  # per-channel temperature

            # ---- big streams (sync/SP queue, in need-order) ----
            xt = consts.tile([C, 64, 2, C], f8)   # pretransposed x1 chunks
            for c in range(4):
                nc.sync.dma_start(
                    out=xt[:, 16 * c : 16 * c + 16],
                    in_=x1t8[:, 4096 * c : 4096 * c + 4096],
                )
            p2x = pads.tile([C, H + 2, PW], f8)   # host-padded fp8 x2
            for c in range(2):
                nc.sync.dma_start(
                    out=p2x[:, 65 * c : 65 * c + 65, :],
                    in_=x2p8[:, 65 * PW * c : 65 * PW * (c + 1)],
                )
            php = pads.tile([C, H + 2, PW], f8)   # conv1 out, reflect-padded

            # ================= phase A: Gram + softmax + L weights ==========
            with (
                tc.tile_pool(name="psG", bufs=1, space="PSUM") as psG,
                tc.tile_pool(name="psS", bufs=1, space="PSUM") as psS,
                tc.tile_pool(name="psW", bufs=2, space="PSUM") as psW,
            ):
                gps = psG.tile([C, C], f32)
                for g in range(64):
                    nc.tensor.matmul(
                        out=gps, lhsT=xt[:, g], rhs=xt[:, g],
                        start=(g == 0), stop=(g == 63),
                        perf_mode=PM.DoubleRow, skip_group_check=True,
                    )

                # diag -> norms -> rn
                gi = attn.tile([C, C], f32)
                nc.vector.tensor_mul(out=gi, in0=gps, in1=idfs)
                diag = attn.tile([C, 1], f32)
                nc.vector.reduce_sum(out=diag, in_=gi, axis=AX.X)
                norm = attn.tile([C, 1], f32)
                nc.scalar.activation(out=norm, in_=diag, func=AF.Sqrt)
                nc.vector.tensor_scalar_max(out=norm, in0=norm, scalar1=EPS)
                rn = attn.tile([C, 1], f32)
                nc.vector.reciprocal(out=rn, in_=norm)

                # S = diag(rn) G diag(rn) via row-scale, transpose, row-scale
                s1 = attn.tile([C, C], f32)
                nc.vector.tensor_scalar_mul(out=s1, in0=gps, scalar1=rn)
                s1t = psS.tile([C, C], f32)
                nc.tensor.transpose(out=s1t, in_=s1, identity=idfs)
                s2 = attn.tile([C, C], f32)
                nc.vector.tensor_scalar_mul(out=s2, in0=s1t, scalar1=rn)

                # E = exp(S * temp) * blockmask ; rinv = 1/rowsum(E)
                e0 = attn.tile([C, C], f32)
                nc.scalar.activation(out=e0, in_=s2, func=AF.Exp, scale=temp_ap)
                em = attn.tile([C, C], f32r)
                nc.vector.tensor_mul(out=em, in0=e0, in1=bmasks)
                rs = attn.tile([C, 1], f32)
                nc.vector.reduce_sum(out=rs, in_=em, axis=AX.X)
                rinv = attn.tile([C, 1], f32)
                nc.vector.reciprocal(out=rinv, in_=rs)

                # L pairs: l8[:, p] = fp8(S_L * E diag(rinv) w1_pair_p).
                # Pair 4 slot 0 is tap8; its slot-1 "zero" k-tile carries the
                # fp8 quantization error of slot 0 (rhs window repeats).
                # w1s pair 4 slot 1 is wsc -> lsc8 with the same trick.
                l8 = attn.tile([C, 5, 2, C], f8)
                lsc8 = attn.tile([C, 2, C], f8)
                for p in range(5):
                    rt = attn.tile([C, 2, C], f32r, name=f"rt{p}", tag="rt")
                    nc.vector.tensor_scalar_mul(out=rt, in0=w1s[:, p], scalar1=rinv)
                    lps = psW.tile([C, 2, C], f32, name=f"lps{p}", tag="lps")
                    nc.tensor.matmul(out=lps, lhsT=em, rhs=rt, start=True, stop=True)
                    if p < 4:
                        nc.scalar.activation(
                            out=l8[:, p], in_=lps, func=AF.Copy, scale=S_L)
                    else:
                        nc.scalar.activation(
                            out=l8[:, 4, 0, :], in_=lps[:, 0, :], func=AF.Copy,
                            scale=S_L)
                        nc.vector.scalar_tensor_tensor(
                            out=l8[:, 4, 1, :], in0=lps[:, 0, :], scalar=S_L,
                            in1=l8[:, 4, 0, :], op0=ALU.mult, op1=ALU.subtract)
                        nc.scalar.activation(
                            out=lsc8[:, 0, :], in_=lps[:, 1, :], func=AF.Copy,
                            scale=S_L)
                        nc.vector.scalar_tensor_tensor(
                            out=lsc8[:, 1, :], in0=lps[:, 1, :], scalar=S_L,
                            in1=lsc8[:, 0, :], op0=ALU.mult, op1=ALU.subtract)

            # first x1 residual chunk before the band loop starts
            state = {}
            state["x1b"] = x1st.tile([C, 2048], bf16, tag="x1b", name="x1b0")
            nc.sync.dma_start(out=state["x1b"], in_=x1b16[:, 0:2048])

            # ============ phases C/D interleaved per 8-row band =============
            # C band b writes php rows 8b+1..8b+8; D band b reads php rows
            # 8b..8b+9 -> needs C bands <= b+1.
            with (
                tc.tile_pool(name="psC", bufs=2, space="PSUM") as psC,
                tc.tile_pool(name="psD", bufs=1, space="PSUM") as psD,
                tc.tile_pool(name="psS2", bufs=1, space="PSUM") as psS2,
            ):
                def emit_C(b):
                    y0 = ROWS * b
                    cps = psC.tile([C, ROWS, C], f32)
                    for h in range(2):
                        for i, (_, _, dy, dx, strd) in enumerate(PAIRS):
                            nc.tensor.matmul(
                                out=cps[:, 4 * h : 4 * h + 4, :],
                                lhsT=l8[:, i],
                                rhs=pair_rhs(p2x, y0 + 4 * h + dy, dx, strd),
                                start=(i == 0), stop=(i == 4),
                                perf_mode=PM.DoubleRow,
                            )
                    nc.scalar.activation(
                        out=php[:, 1 + y0 : 1 + y0 + ROWS, 1 : 1 + C],
                        in_=cps, func=AF.Prelu, bias=b1_ap,
                        scale=S_PH / S_L, alpha=SLOPE,
                    )
                    # incremental reflect pad of the left/right columns
                    nc.vector.tensor_copy(
                        out=php[:, 1 + y0 : 1 + y0 + ROWS, 0:1],
                        in_=php[:, 1 + y0 : 1 + y0 + ROWS, 2:3])
                    nc.vector.tensor_copy(
                        out=php[:, 1 + y0 : 1 + y0 + ROWS, PW - 1 : PW],
                        in_=php[:, 1 + y0 : 1 + y0 + ROWS, PW - 3 : PW - 2])
                    if b == 0:
                        nc.vector.tensor_copy(out=php[:, 0:1, :], in_=php[:, 2:3, :])
                    if b == NB - 1:
                        nc.vector.tensor_copy(
                            out=php[:, H + 1 : H + 2, :], in_=php[:, H - 1 : H, :])

                def emit_D(b):
                    y0 = ROWS * b
                    if b % 2 == 0:
                        otile = x1st.tile([C, 2048], bf16, tag="otile")
                        state["otile"] = otile
                        if b + 2 < NB:  # prefetch next residual chunk
                            nx = x1st.tile([C, 2048], bf16, tag="x1b")
                            nc.sync.dma_start(
                                out=nx, in_=x1b16[:, (b + 2) * 1024 : (b + 4) * 1024])
                            state["x1b_next"] = nx
                    x1b = state["x1b"][:, (b % 2) * 1024 : (b % 2 + 1) * 1024]
                    x1b = x1b.rearrange("p (a b) -> p a b", a=ROWS)
                    dps = psD.tile([C, ROWS, C], f32)
                    for h in range(2):
                        for i, (_, _, dy, dx, strd) in enumerate(PAIRS):
                            nc.tensor.matmul(
                                out=dps[:, 4 * h : 4 * h + 4, :],
                                lhsT=w2s[:, i],
                                rhs=pair_rhs(php, y0 + 4 * h + dy, dx, strd),
                                start=(i == 0), stop=(i == 4),
                                perf_mode=PM.DoubleRow,
                            )
                    sps = psS2.tile([C, ROWS, C], f32)
                    for h in range(2):
                        nc.tensor.matmul(
                            out=sps[:, 4 * h : 4 * h + 4, :],
                            lhsT=lsc8,
                            rhs=pair_rhs(p2x, y0 + 4 * h + 1, 1, 0),
                            start=True, stop=True,
                            perf_mode=PM.DoubleRow,
                        )
                    h3 = bands.tile([C, ROWS, C], bf16)
                    nc.scalar.activation(
                        out=h3, in_=dps, func=AF.Prelu, bias=b2_ap,
                        scale=1.0 / (S_2 * S_PH), alpha=SLOPE,
                    )
                    # epilogue: out = h3 + sps/S_L + bsc + x1
                    t = bands.tile([C, ROWS, C], bf16, tag="t")
                    nc.vector.scalar_tensor_tensor(
                        out=t, in0=sps, scalar=1.0 / S_L, in1=x1b,
                        op0=ALU.mult, op1=ALU.add)
                    ob = state["otile"][:, (b % 2) * 1024 : (b % 2 + 1) * 1024]
                    ob = ob.rearrange("p (a b) -> p a b", a=ROWS)
                    nc.vector.scalar_tensor_tensor(
                        out=ob, in0=t, scalar=bsc_ap, in1=h3,
                        op0=ALU.add, op1=ALU.add)
                    if b % 2 == 1:
                        nc.sync.dma_start(
                            out=out[:, (b - 1) * 1024 : (b + 1) * 1024],
                            in_=state["otile"])
                        if "x1b_next" in state:
                            state["x1b"] = state.pop("x1b_next")

                for k in range(NB + 1):
                    if k < NB:
                        emit_C(k)
                    if k >= 1:
                        emit_D(k - 1)

    nc.compile()
    return nc


def _prep_consts(temperature, conv1_w, conv1_b, conv2_w, conv2_b,
                 dw_w, dw_b, sc_w, sc_b):
    f32 = np.float32
    f8 = ml_dtypes.float8_e4m3
    conv1_w = np.asarray(conv1_w, f32)
    conv2_w = np.asarray(conv2_w, f32)
    dw_w = np.asarray(dw_w, f32)
    sc_w = np.asarray(sc_w, f32)

    # conv1 taps as lhsT: wc1[ci, t, co] = conv1_w[co, ci, dy, dx], arranged
    # in DoubleRow pair order; pair-4 slot 1 is the shortcut weight.
    wc1 = conv1_w.transpose(1, 2, 3, 0).reshape(C, 9, C)
    w1p = np.empty((C, 5, 2, C), f32)
    for i, (ta, tb, _, _, _) in enumerate(PAIRS[:4]):
        w1p[:, i, 0] = wc1[:, ta]
        w1p[:, i, 1] = wc1[:, tb]
    w1p[:, 4, 0] = wc1[:, 8]
    w1p[:, 4, 1] = sc_w[:, :, 0, 0].T

    # fused conv2+dw taps (scaled to fp8 range), same pair order; pair-4
    # slot 1 carries the fp8 quantization error of tap 8.
    A2 = conv2_w[:, :, 0, 0]                      # [co, ci]
    Dw = dw_w[:, 0, :, :].reshape(C, 9)           # [co, t]
    wc2 = np.einsum("oc,ot->tco", A2, Dw) * S_2   # [t, ci, co]
    w2p = np.empty((C, 5, 2, C), f8)
    for i, (ta, tb, _, _, _) in enumerate(PAIRS[:4]):
        w2p[:, i, 0] = wc2[ta].astype(f8)
        w2p[:, i, 1] = wc2[tb].astype(f8)
    t8q = wc2[8].astype(f8)
    w2p[:, 4, 0] = t8q
    w2p[:, 4, 1] = (wc2[8] - t8q.astype(f32)).astype(f8)

    b2p = np.asarray(dw_b, f32) + np.asarray(conv2_b, f32) * Dw.sum(axis=1)
    temp_b = np.repeat(np.asarray(temperature, f32).reshape(HEADS), HEAD_C)
    sclv = np.ascontiguousarray(np.stack(
        [np.asarray(conv1_b, f32) * S_PH, b2p, np.asarray(sc_b, f32), temp_b],
        axis=1))
    bmaskv = np.kron(np.eye(HEADS, dtype=f32), np.ones((HEAD_C, HEAD_C), f32))
    return dict(
        w1p=np.ascontiguousarray(w1p), w2p=np.ascontiguousarray(w2p),
        sclv=sclv, bmask=np.ascontiguousarray(bmaskv),
        idf=np.eye(C, dtype=f32),
    )


def kernel(
    x1, x2, temperature, conv1_w, conv1_b, conv2_w, conv2_b, dw_w, dw_b, sc_w, sc_b
):
    from concourse.bass_utils import run_bass_kernel_spmd

    if "nc" not in _cache:
        _cache["nc"] = _build_program()
    nc = _cache["nc"]

    f8 = ml_dtypes.float8_e4m3
    bf16 = ml_dtypes.bfloat16
    x1 = np.ascontiguousarray(np.asarray(x1, np.float32))
    x2 = np.ascontiguousarray(np.asarray(x2, np.float32))
    consts = _prep_consts(
        temperature, conv1_w, conv1_b, conv2_w, conv2_b, dw_w, dw_b, sc_w, sc_b)

    in_maps = []
    for b in range(B):
        x1f = x1[b].reshape(C, HW)
        # pretransposed fp8 x1 in DoubleRow chunk order [p, g, j, c] with
        # spatial index = 256g + 128j + p
        x1t = x1f.T.reshape(64, 2, 128, C).transpose(2, 0, 1, 3).reshape(C, HW)
        x2p = np.zeros((C, H + 2, PW), np.float32)
        x2p[:, 1 : H + 1, 1 : W + 1] = x2[b].reshape(C, H, W)
        m = dict(consts)
        m["x1t8"] = np.ascontiguousarray(x1t.astype(f8))
        m["x2p8"] = np.ascontiguousarray(x2p.astype(f8).reshape(C, (H + 2) * PW))
        m["x1b16"] = np.ascontiguousarray(x1f.astype(bf16))
        in_maps.append(m)

    res = run_bass_kernel_spmd(nc, in_maps, core_ids=list(range(B)))
    outs = [
        res.results[b]["out"].astype(np.float32).reshape(C, H, W)
        for b in range(B)
    ]
    return np.stack(outs, axis=0)


# revision 5
# speedup vs baseline: 2.1730x; 1.0360x over previous
"""Trainium2 Bass kernel for nn_Cross_Attention_18425409700231.

Per-sample channel attention (16 heads x 8 channels, L2-normalized over
spatial, softmax over in-head channels) followed by a conv block
(3x3 conv -> LeakyReLU -> 1x1 conv -> reflect-pad depthwise 3x3 ->
LeakyReLU, plus 1x1 shortcut) and a residual add.

Sharding: data-parallel over batch B=8 -> one sample per NeuronCore.

Device algorithm per core (sample b):
  A. Gram matrix G = x1 @ x1^T from a HOST-pretransposed fp8 copy of x1
     ([spatial, chan] layout, so no PE transposes), accumulated with fp8
     DoubleRow matmuls (two 128-deep k-tiles per instruction, 0.5
     cycles/row).  Norms from diag(G); S = rn*G*rn; E = exp(S*temp)
     masked to the 16 block-diagonal 8x8 head blocks; rinv = 1/rowsum.
  B. Fused attention+conv weights L_t = E diag(rinv) w1_t and
     Lsc = E diag(rinv) wsc, cast to fp8 (x64 scale) and packed in
     DoubleRow tap-pairs; the shortcut/odd-tap zero k-tiles carry fp8
     quantization-error compensation terms instead of zeros.
  C. conv1 3x3 on host-zero-padded fp8 x2: per 8-row band, 10 DoubleRow
     matmuls (2 taps each) into 2 PSUM banks; two half-band ACT passes
     do bias+LeakyReLU into the reflect-padded fp8 conv1 buffer.
  D. conv2(1x1)+depthwise 3x3 fused into 9 taps (host-packed fp8
     DoubleRow pairs) + the shortcut pair; bias+LeakyReLU on ACT;
     epilogue on DVE in bf16 (sc_b is pre-folded into the host-shipped
     x1 residual); out streamed to DRAM as bf16, upcast on host.
  C/D interleave per band with the D half that needs the freshest conv1
  rows emitted last, so the PE never waits on the ACT passes.
"""

import numpy as np
import ml_dtypes

B, C, H, W = 8, 128, 128, 128
HW = H * W
HEADS, HEAD_C = 16, 8
SLOPE = 0.2
PW = W + 2          # padded width
ROWS = 8            # band height
NB = H // ROWS      # 16 bands
S_L = 64.0          # fp8 scale for attention-fused conv1/shortcut weights
S_PH = 4.0          # fp8 scale for the conv1 activation buffer
S_2 = 256.0         # fp8 scale for fused conv2*dw weights

# DoubleRow tap pairs: (tap_a, tap_b, window row offset dy, col offset dx,
# elem stride between the two windows).  Taps are dy*3+dx.
PAIRS = [
    (0, 1, 0, 0, 1),      # (0,0)+(0,1): shift right by 1
    (2, 5, 0, 2, PW),     # (0,2)+(1,2): shift down by 1 row
    (3, 4, 1, 0, 1),      # (1,0)+(1,1)
    (6, 7, 2, 0, 1),      # (2,0)+(2,1)
    (8, -1, 2, 2, 0),     # (2,2) + its fp8 compensation (same window)
]

_cache = {}


def _build_program():
    import concourse.bass as bass
    import concourse.tile as tile
    import concourse.mybir as mybir
    from concourse import bacc

    dt = mybir.dt
    f32, f32r, bf16, f8 = dt.float32, dt.float32r, dt.bfloat16, dt.float8e4
    AF = mybir.ActivationFunctionType
    ALU = mybir.AluOpType
    AX = mybir.AxisListType
    PM = mybir.MatmulPerfMode

    nc = bacc.Bacc("TRN2", num_devices=8)

    x1t8 = nc.dram_tensor("x1t8", [C, HW], f8, kind="ExternalInput").ap()
    x2p8 = nc.dram_tensor("x2p8", [C, (H + 2) * PW], f8, kind="ExternalInput").ap()
    x1b16 = nc.dram_tensor("x1b16", [C, HW], bf16, kind="ExternalInput").ap()
    w1p = nc.dram_tensor("w1p", [C, 5, 2, C], f32r, kind="ExternalInput").ap()
    w2p = nc.dram_tensor("w2p", [C, 5, 2, C], f8, kind="ExternalInput").ap()
    sclv = nc.dram_tensor("sclv", [C, 3], f32, kind="ExternalInput").ap()
    bmask = nc.dram_tensor("bmask", [C, C], f32, kind="ExternalInput").ap()
    idf = nc.dram_tensor("idf", [C, C], f32, kind="ExternalInput").ap()
    out = nc.dram_tensor("out", [C, HW], bf16, kind="ExternalOutput").ap()

    def pair_rhs(buf, y, x, strd):
        """[C, 2, 4, C] DoubleRow moving AP: two 3x3-tap windows of a padded
        [C, 130, 130] buffer, 4 rows each, dim1 = the tap pair."""
        base = buf[:, y : y + 4, x : x + C].unsqueeze(1)
        if strd == 0:
            return base.broadcast_to([C, 2, 4, C])
        ap = [list(p) for p in base.ap]
        ap[1] = [strd, 2]
        return bass.AP(tensor=base.tensor, offset=base.offset, ap=ap)

    with tile.TileContext(nc) as tc:
        with (
            tc.tile_pool(name="consts", bufs=1) as consts,
            tc.tile_pool(name="pads", bufs=1) as pads,
            tc.tile_pool(name="attn", bufs=1) as attn,
            tc.tile_pool(name="bands", bufs=2) as bands,
            tc.tile_pool(name="x1st", bufs=2) as x1st,
        ):
            # ---- constants to SBUF (Pool SWDGE queue; off the main stream) ----
            w1s = consts.tile([C, 5, 2, C], f32r)
            nc.gpsimd.dma_start(out=w1s, in_=w1p)
            w2s = consts.tile([C, 5, 2, C], f8)
            nc.gpsimd.dma_start(out=w2s, in_=w2p)
            scls = consts.tile([C, 3], f32)
            nc.gpsimd.dma_start(out=scls, in_=sclv)
            bmasks = consts.tile([C, C], f32)
            nc.gpsimd.dma_start(out=bmasks, in_=bmask)
            idfs = consts.tile([C, C], f32)
            nc.gpsimd.dma_start(out=idfs, in_=idf)
            b1_ap = scls[:, 0:1]    # S_PH * conv1_b
            b2_ap = scls[:, 1:2]    # dw_b + conv2_b * sum(dw)
            temp_ap = scls[:, 2:3]  # per-channel temperature

            # ---- big streams (sync/SP queue, in need-order) ----
            xt = consts.tile([C, 64, 2, C], f8)   # pretransposed x1 chunks
            for c in range(4):
                nc.sync.dma_start(
                    out=xt[:, 16 * c : 16 * c + 16],
                    in_=x1t8[:, 4096 * c : 4096 * c + 4096],
                )
            p2x = pads.tile([C, H + 2, PW], f8)   # host-padded fp8 x2
            for c in range(2):
                nc.sync.dma_start(
                    out=p2x[:, 65 * c : 65 * c + 65, :],
                    in_=x2p8[:, 65 * PW * c : 65 * PW * (c + 1)],
                )
            php = pads.tile([C, H + 2, PW], f8)   # conv1 out, reflect-padded

            # ================= phase A: Gram + softmax + L weights ==========
            with (
                tc.tile_pool(name="psG", bufs=1, space="PSUM") as psG,
                tc.tile_pool(name="psS", bufs=1, space="PSUM") as psS,
                tc.tile_pool(name="psW", bufs=2, space="PSUM") as psW,
            ):
                gps = psG.tile([C, C], f32)
                for g in range(64):
                    nc.tensor.matmul(
                        out=gps, lhsT=xt[:, g], rhs=xt[:, g],
                        start=(g == 0), stop=(g == 63),
                        perf_mode=PM.DoubleRow, skip_group_check=True,
                    )

                # diag -> norms -> rn  (norms ~ sqrt(16384); eps clamp unneeded)
                gi = attn.tile([C, C], f32)
                nc.vector.tensor_mul(out=gi, in0=gps, in1=idfs)
                diag = attn.tile([C, 1], f32)
                nc.vector.reduce_sum(out=diag, in_=gi, axis=AX.X)
                norm = attn.tile([C, 1], f32)
                nc.scalar.activation(out=norm, in_=diag, func=AF.Sqrt)
                rn = attn.tile([C, 1], f32)
                nc.vector.reciprocal(out=rn, in_=norm)

                # S = diag(rn) G diag(rn) via row-scale, transpose, row-scale
                s1 = attn.tile([C, C], f32)
                nc.vector.tensor_scalar_mul(out=s1, in0=gps, scalar1=rn)
                s1t = psS.tile([C, C], f32)
                nc.tensor.transpose(out=s1t, in_=s1, identity=idfs)
                s2 = attn.tile([C, C], f32)
                nc.vector.tensor_scalar_mul(out=s2, in0=s1t, scalar1=rn)

                # E = exp(S * temp) * blockmask ; rinv = 1/rowsum(E)
                e0 = attn.tile([C, C], f32)
                nc.scalar.activation(out=e0, in_=s2, func=AF.Exp, scale=temp_ap)
                em = attn.tile([C, C], f32r)
                nc.vector.tensor_mul(out=em, in0=e0, in1=bmasks)
                rs = attn.tile([C, 1], f32)
                nc.vector.reduce_sum(out=rs, in_=em, axis=AX.X)
                rinv = attn.tile([C, 1], f32)
                nc.vector.reciprocal(out=rinv, in_=rs)

                # L pairs: l8[:, p] = fp8(S_L * E diag(rinv) w1_pair_p).
                # Pair 4 slot 0 is tap8; its slot-1 "zero" k-tile carries the
                # fp8 quantization error of slot 0 (rhs window repeats).
                # w1s pair 4 slot 1 is wsc -> lsc8 with the same trick.
                l8 = attn.tile([C, 5, 2, C], f8)
                lsc8 = attn.tile([C, 2, C], f8)
                for p in range(5):
                    rt = attn.tile([C, 2, C], f32r, name=f"rt{p}", tag="rt")
                    nc.vector.tensor_scalar_mul(out=rt, in0=w1s[:, p], scalar1=rinv)
                    lps = psW.tile([C, 2, C], f32, name=f"lps{p}", tag="lps")
                    nc.tensor.matmul(out=lps, lhsT=em, rhs=rt, start=True, stop=True)
                    if p < 4:
                        nc.scalar.activation(
                            out=l8[:, p], in_=lps, func=AF.Copy, scale=S_L)
                    else:
                        nc.scalar.activation(
                            out=l8[:, 4, 0, :], in_=lps[:, 0, :], func=AF.Copy,
                            scale=S_L)
                        nc.vector.scalar_tensor_tensor(
                            out=l8[:, 4, 1, :], in0=lps[:, 0, :], scalar=S_L,
                            in1=l8[:, 4, 0, :], op0=ALU.mult, op1=ALU.subtract)
                        nc.scalar.activation(
                            out=lsc8[:, 0, :], in_=lps[:, 1, :], func=AF.Copy,
                            scale=S_L)
                        nc.vector.scalar_tensor_tensor(
                            out=lsc8[:, 1, :], in0=lps[:, 1, :], scalar=S_L,
                            in1=lsc8[:, 0, :], op0=ALU.mult, op1=ALU.subtract)

            # first x1 residual chunk before the band loop starts
            state = {}
            state["x1b"] = x1st.tile([C, 2048], bf16, tag="x1b", name="x1b0")
            nc.sync.dma_start(out=state["x1b"], in_=x1b16[:, 0:2048])

            # ============ phases C/D interleaved per 8-row band =============
            # C band b writes php rows 8b+1..8b+8; D band b's first matmul
            # half reads php rows <= 8b+5 (band b itself), its second half
            # rows <= 8b+9 (band b+1's first ACT half-pass).
            with (
                tc.tile_pool(name="psC", bufs=4, space="PSUM") as psC,
                tc.tile_pool(name="psD", bufs=1, space="PSUM") as psD,
                tc.tile_pool(name="psS2", bufs=1, space="PSUM") as psS2,
            ):
                def emit_C(b):
                    y0 = ROWS * b
                    halves = []
                    for h in range(2):
                        cps = psC.tile([C, 4, C], f32, name=f"cps{b}_{h}", tag="cps")
                        for i, (_, _, dy, dx, strd) in enumerate(PAIRS):
                            nc.tensor.matmul(
                                out=cps,
                                lhsT=l8[:, i],
                                rhs=pair_rhs(p2x, y0 + 4 * h + dy, dx, strd),
                                start=(i == 0), stop=(i == 4),
                                perf_mode=PM.DoubleRow,
                            )
                        halves.append(cps)
                    for h, cps in enumerate(halves):
                        r0 = 1 + y0 + 4 * h
                        nc.scalar.activation(
                            out=php[:, r0 : r0 + 4, 1 : 1 + C],
                            in_=cps, func=AF.Prelu, bias=b1_ap,
                            scale=S_PH / S_L, alpha=SLOPE,
                        )
                        # incremental reflect pad of the left/right columns
                        nc.gpsimd.tensor_copy(
                            out=php[:, r0 : r0 + 4, 0:1],
                            in_=php[:, r0 : r0 + 4, 2:3])
                        nc.gpsimd.tensor_copy(
                            out=php[:, r0 : r0 + 4, PW - 1 : PW],
                            in_=php[:, r0 : r0 + 4, PW - 3 : PW - 2])
                    if b == 0:
                        nc.gpsimd.tensor_copy(out=php[:, 0:1, :], in_=php[:, 2:3, :])
                    if b == NB - 1:
                        nc.gpsimd.tensor_copy(
                            out=php[:, H + 1 : H + 2, :], in_=php[:, H - 1 : H, :])

                def emit_D(b):
                    y0 = ROWS * b
                    if b % 2 == 0:
                        otile = x1st.tile([C, 2048], bf16, tag="otile")
                        state["otile"] = otile
                        if b + 2 < NB:  # prefetch next residual chunk
                            nx = x1st.tile([C, 2048], bf16, tag="x1b")
                            nc.sync.dma_start(
                                out=nx, in_=x1b16[:, (b + 2) * 1024 : (b + 4) * 1024])
                            state["x1b_next"] = nx
                    x1b = state["x1b"][:, (b % 2) * 1024 : (b % 2 + 1) * 1024]
                    x1b = x1b.rearrange("p (a b) -> p a b", a=ROWS)
                    # order: dps h0 (band-b php rows only), shortcut, dps h1
                    # (needs band b+1's first ACT half-pass) last.
                    dps = psD.tile([C, ROWS, C], f32)
                    for h in range(2):
                        if h == 1:
                            sps = psS2.tile([C, ROWS, C], f32)
                            for g in range(2):
                                nc.tensor.matmul(
                                    out=sps[:, 4 * g : 4 * g + 4, :],
                                    lhsT=lsc8,
                                    rhs=pair_rhs(p2x, y0 + 4 * g + 1, 1, 0),
                                    start=True, stop=True,
                                    perf_mode=PM.DoubleRow,
                                )
                        for i, (_, _, dy, dx, strd) in enumerate(PAIRS):
                            nc.tensor.matmul(
                                out=dps[:, 4 * h : 4 * h + 4, :],
                                lhsT=w2s[:, i],
                                rhs=pair_rhs(php, y0 + 4 * h + dy, dx, strd),
                                start=(i == 0), stop=(i == 4),
                                perf_mode=PM.DoubleRow,
                            )
                    h3 = bands.tile([C, ROWS, C], bf16)
                    nc.scalar.activation(
                        out=h3, in_=dps, func=AF.Prelu, bias=b2_ap,
                        scale=1.0 / (S_2 * S_PH), alpha=SLOPE,
                    )
                    # epilogue: out = h3 + sps/S_L + (x1 + sc_b)  [sc_b folded
                    # into x1b on the host]
                    t = bands.tile([C, ROWS, C], bf16, tag="t")
                    nc.vector.scalar_tensor_tensor(
                        out=t, in0=sps, scalar=1.0 / S_L, in1=x1b,
                        op0=ALU.mult, op1=ALU.add)
                    ob = state["otile"][:, (b % 2) * 1024 : (b % 2 + 1) * 1024]
                    ob = ob.rearrange("p (a b) -> p a b", a=ROWS)
                    nc.vector.tensor_add(out=ob, in0=t, in1=h3)
                    if b % 2 == 1:
                        nc.sync.dma_start(
                            out=out[:, (b - 1) * 1024 : (b + 1) * 1024],
                            in_=state["otile"])
                        if "x1b_next" in state:
                            state["x1b"] = state.pop("x1b_next")

                for k in range(NB + 1):
                    if k < NB:
                        emit_C(k)
                    if k >= 1:
                        emit_D(k - 1)

    nc.compile()
    return nc


def _prep_consts(temperature, conv1_w, conv1_b, conv2_w, conv2_b,
                 dw_w, dw_b, sc_w, sc_b):
    f32 = np.float32
    f8 = ml_dtypes.float8_e4m3
    conv1_w = np.asarray(conv1_w, f32)
    conv2_w = np.asarray(conv2_w, f32)
    dw_w = np.asarray(dw_w, f32)
    sc_w = np.asarray(sc_w, f32)

    # conv1 taps as lhsT: wc1[ci, t, co] = conv1_w[co, ci, dy, dx], arranged
    # in DoubleRow pair order; pair-4 slot 1 is the shortcut weight.
    wc1 = conv1_w.transpose(1, 2, 3, 0).reshape(C, 9, C)
    w1p = np.empty((C, 5, 2, C), f32)
    for i, (ta, tb, _, _, _) in enumerate(PAIRS[:4]):
        w1p[:, i, 0] = wc1[:, ta]
        w1p[:, i, 1] = wc1[:, tb]
    w1p[:, 4, 0] = wc1[:, 8]
    w1p[:, 4, 1] = sc_w[:, :, 0, 0].T

    # fused conv2+dw taps (scaled to fp8 range), same pair order; pair-4
    # slot 1 carries the fp8 quantization error of tap 8.
    A2 = conv2_w[:, :, 0, 0]                      # [co, ci]
    Dw = dw_w[:, 0, :, :].reshape(C, 9)           # [co, t]
    wc2 = np.einsum("oc,ot->tco", A2, Dw) * S_2   # [t, ci, co]
    w2p = np.empty((C, 5, 2, C), f8)
    for i, (ta, tb, _, _, _) in enumerate(PAIRS[:4]):
        w2p[:, i, 0] = wc2[ta].astype(f8)
        w2p[:, i, 1] = wc2[tb].astype(f8)
    t8q = wc2[8].astype(f8)
    w2p[:, 4, 0] = t8q
    w2p[:, 4, 1] = (wc2[8] - t8q.astype(f32)).astype(f8)

    b2p = np.asarray(dw_b, f32) + np.asarray(conv2_b, f32) * Dw.sum(axis=1)
    temp_b = np.repeat(np.asarray(temperature, f32).reshape(HEADS), HEAD_C)
    sclv = np.ascontiguousarray(np.stack(
        [np.asarray(conv1_b, f32) * S_PH, b2p, temp_b], axis=1))
    bmaskv = np.kron(np.eye(HEADS, dtype=f32), np.ones((HEAD_C, HEAD_C), f32))
    return dict(
        w1p=np.ascontiguousarray(w1p), w2p=np.ascontiguousarray(w2p),
        sclv=sclv, bmask=np.ascontiguousarray(bmaskv),
        idf=np.eye(C, dtype=f32),
    )


def kernel(
    x1, x2, temperature, conv1_w, conv1_b, conv2_w, conv2_b, dw_w, dw_b, sc_w, sc_b
):
    from concourse.bass_utils import run_bass_kernel_spmd

    if "nc" not in _cache:
        _cache["nc"] = _build_program()
    nc = _cache["nc"]

    f8 = ml_dtypes.float8_e4m3
    bf16 = ml_dtypes.bfloat16
    x1 = np.ascontiguousarray(np.asarray(x1, np.float32))
    x2 = np.ascontiguousarray(np.asarray(x2, np.float32))
    consts = _prep_consts(
        temperature, conv1_w, conv1_b, conv2_w, conv2_b, dw_w, dw_b, sc_w, sc_b)
    scb = np.asarray(sc_b, np.float32)[:, None]

    in_maps = []
    for b in range(B):
        x1f = x1[b].reshape(C, HW)
        # pretransposed fp8 x1 in DoubleRow chunk order [p, g, j, c] with
        # spatial index = 256g + 128j + p
        x1t = x1f.T.reshape(64, 2, 128, C).transpose(2, 0, 1, 3).reshape(C, HW)
        x2p = np.zeros((C, H + 2, PW), np.float32)
        x2p[:, 1 : H + 1, 1 : W + 1] = x2[b].reshape(C, H, W)
        m = dict(consts)
        m["x1t8"] = np.ascontiguousarray(x1t.astype(f8))
        m["x2p8"] = np.ascontiguousarray(x2p.astype(f8).reshape(C, (H + 2) * PW))
        m["x1b16"] = np.ascontiguousarray((x1f + scb).astype(bf16))
        in_maps.append(m)

    res = run_bass_kernel_spmd(nc, in_maps, core_ids=list(range(B)))
    outs = [
        res.results[b]["out"].astype(np.float32).reshape(C, H, W)
        for b in range(B)
    ]
    return np.stack(outs, axis=0)


# revision 6
# speedup vs baseline: 2.5836x; 1.1889x over previous
"""Trainium2 Bass kernel for nn_Cross_Attention_18425409700231.

Per-sample channel attention (16 heads x 8 channels, L2-normalized over
spatial, softmax over in-head channels) followed by a conv block
(3x3 conv -> LeakyReLU -> 1x1 conv -> reflect-pad depthwise 3x3 ->
LeakyReLU, plus 1x1 shortcut) and a residual add.

Sharding: data-parallel over batch B=8 -> one sample per NeuronCore.

Device algorithm per core (sample b):
  A. Gram matrix G = x1 @ x1^T from a HOST-pretransposed fp8 copy of x1
     ([spatial, chan] layout, so no PE transposes), accumulated with fp8
     DoubleRow matmuls (two 128-deep k-tiles per instruction, 0.5
     cycles/row).  Norms from diag(G); S = rn*G*rn; E = exp(S*temp)
     masked to the 16 block-diagonal 8x8 head blocks; rinv = 1/rowsum.
  B. Fused attention+conv weights L_t = E diag(rinv) w1_t and
     Lsc = E diag(rinv) wsc, cast to fp8 (x64 scale) and packed in
     DoubleRow tap-pairs; the shortcut/odd-tap zero k-tiles carry fp8
     quantization-error compensation terms instead of zeros.
  C. conv1 3x3 on host-zero-padded fp8 x2: per 8-row band, 10 DoubleRow
     matmuls (2 taps each) into 2 PSUM banks; two half-band ACT passes
     do bias+LeakyReLU into the reflect-padded fp8 conv1 buffer.
  D. conv2(1x1)+depthwise 3x3 fused into 9 taps (host-packed fp8
     DoubleRow pairs) + the shortcut pair; bias+LeakyReLU on ACT;
     epilogue on DVE in bf16 (sc_b is pre-folded into the host-shipped
     x1 residual); out streamed to DRAM as bf16, upcast on host.
  C/D interleave per band with the D half that needs the freshest conv1
  rows emitted last, so the PE never waits on the ACT passes.  DMAs are
  issued on one queue in need-order (the DMA engines are a serial
  resource): packed small consts, conv1 weights, x1t chunks, first x2
  chunk, conv2 weights, rest of x2, then the x1-residual/out stream.
"""

import numpy as np
import ml_dtypes

B, C, H, W = 8, 128, 128, 128
HW = H * W
HEADS, HEAD_C = 16, 8
SLOPE = 0.2
PW = W + 2          # padded width
ROWS = 8            # band height
NB = H // ROWS      # 16 bands
S_L = 64.0          # fp8 scale for attention-fused conv1/shortcut weights
S_PH = 4.0          # fp8 scale for the conv1 activation buffer
S_2 = 256.0         # fp8 scale for fused conv2*dw weights

# DoubleRow tap pairs: (tap_a, tap_b, window row offset dy, col offset dx,
# elem stride between the two windows).  Taps are dy*3+dx.
PAIRS = [
    (0, 1, 0, 0, 1),      # (0,0)+(0,1): shift right by 1
    (2, 5, 0, 2, PW),     # (0,2)+(1,2): shift down by 1 row
    (3, 4, 1, 0, 1),      # (1,0)+(1,1)
    (6, 7, 2, 0, 1),      # (2,0)+(2,1)
    (8, -1, 2, 2, 0),     # (2,2) + its fp8 compensation (same window)
]

_cache = {}


def _build_program():
    import concourse.bass as bass
    import concourse.tile as tile
    import concourse.mybir as mybir
    from concourse import bacc

    dt = mybir.dt
    f32, f32r, bf16, f8 = dt.float32, dt.float32r, dt.bfloat16, dt.float8e4
    AF = mybir.ActivationFunctionType
    ALU = mybir.AluOpType
    AX = mybir.AxisListType
    PM = mybir.MatmulPerfMode

    nc = bacc.Bacc("TRN2", num_devices=8)

    x1t8 = nc.dram_tensor("x1t8", [C, HW], f8, kind="ExternalInput").ap()
    x2p8 = nc.dram_tensor("x2p8", [C, (H + 2) * PW], f8, kind="ExternalInput").ap()
    x1b16 = nc.dram_tensor("x1b16", [C, HW], bf16, kind="ExternalInput").ap()
    w1p = nc.dram_tensor("w1p", [C, 5, 2, C], f32r, kind="ExternalInput").ap()
    w2p = nc.dram_tensor("w2p", [C, 5, 2, C], f8, kind="ExternalInput").ap()
    # packed small consts: [0:3]=per-channel vectors, [3:131]=bmask,
    # [131:259]=identity
    cpack = nc.dram_tensor("cpack", [C, 259], f32, kind="ExternalInput").ap()
    out = nc.dram_tensor("out", [C, HW], bf16, kind="ExternalOutput").ap()

    def pair_rhs(buf, y, x, strd):
        """[C, 2, 4, C] DoubleRow moving AP: two 3x3-tap windows of a padded
        [C, 130, 130] buffer, 4 rows each, dim1 = the tap pair."""
        base = buf[:, y : y + 4, x : x + C].unsqueeze(1)
        if strd == 0:
            return base.broadcast_to([C, 2, 4, C])
        ap = [list(p) for p in base.ap]
        ap[1] = [strd, 2]
        return bass.AP(tensor=base.tensor, offset=base.offset, ap=ap)

    with tile.TileContext(nc) as tc:
        with (
            tc.tile_pool(name="consts", bufs=1) as consts,
            tc.tile_pool(name="pads", bufs=1) as pads,
            tc.tile_pool(name="attn", bufs=1) as attn,
            tc.tile_pool(name="bands", bufs=2) as bands,
            tc.tile_pool(name="x1st", bufs=2) as x1st,
            tc.tile_pool(name="ost", bufs=2) as ost,
        ):
            # ---- all DMAs on the sync queue, in need-order ----
            cpk = consts.tile([C, 259], f32)
            nc.sync.dma_start(out=cpk, in_=cpack)
            scls = cpk[:, 0:3]
            bmasks = cpk[:, 3:131]
            idfs = cpk[:, 131:259]
            b1_ap = scls[:, 0:1]    # S_PH * conv1_b
            b2_ap = scls[:, 1:2]    # dw_b + conv2_b * sum(dw)
            temp_ap = scls[:, 2:3]  # per-channel temperature

            w1s = consts.tile([C, 5, 2, C], f32r)
            nc.sync.dma_start(out=w1s, in_=w1p)

            xt = consts.tile([C, 64, 2, C], f8)   # pretransposed x1 chunks
            for c in range(4):
                nc.sync.dma_start(
                    out=xt[:, 16 * c : 16 * c + 16],
                    in_=x1t8[:, 4096 * c : 4096 * c + 4096],
                )
            p2x = pads.tile([C, H + 2, PW], f8)   # host-padded fp8 x2
            nc.sync.dma_start(
                out=p2x[:, 0:65, :], in_=x2p8[:, 0 : 65 * PW])
            w2s = consts.tile([C, 5, 2, C], f8)
            nc.sync.dma_start(out=w2s, in_=w2p)
            nc.sync.dma_start(
                out=p2x[:, 65:130, :], in_=x2p8[:, 65 * PW : 130 * PW])
            php = pads.tile([C, H + 2, PW], f8)   # conv1 out, reflect-padded

            # ================= phase A: Gram + softmax + L weights ==========
            with (
                tc.tile_pool(name="psG", bufs=1, space="PSUM") as psG,
                tc.tile_pool(name="psS", bufs=1, space="PSUM") as psS,
                tc.tile_pool(name="psW", bufs=2, space="PSUM") as psW,
            ):
                gps = psG.tile([C, C], f32)
                for g in range(64):
                    nc.tensor.matmul(
                        out=gps, lhsT=xt[:, g], rhs=xt[:, g],
                        start=(g == 0), stop=(g == 63),
                        perf_mode=PM.DoubleRow, skip_group_check=True,
                    )

                # diag -> norms -> rn  (norms ~ sqrt(16384); eps clamp unneeded)
                gi = attn.tile([C, C], f32)
                nc.vector.tensor_mul(out=gi, in0=gps, in1=idfs)
                diag = attn.tile([C, 1], f32)
                nc.vector.reduce_sum(out=diag, in_=gi, axis=AX.X)
                norm = attn.tile([C, 1], f32)
                nc.scalar.activation(out=norm, in_=diag, func=AF.Sqrt)
                rn = attn.tile([C, 1], f32)
                nc.vector.reciprocal(out=rn, in_=norm)

                # S = diag(rn) G diag(rn) via row-scale, transpose, row-scale
                s1 = attn.tile([C, C], f32)
                nc.vector.tensor_scalar_mul(out=s1, in0=gps, scalar1=rn)
                s1t = psS.tile([C, C], f32)
                nc.tensor.transpose(out=s1t, in_=s1, identity=idfs)
                s2 = attn.tile([C, C], f32)
                nc.vector.tensor_scalar_mul(out=s2, in0=s1t, scalar1=rn)

                # E = exp(S * temp) * blockmask ; rinv = 1/rowsum(E)
                e0 = attn.tile([C, C], f32)
                nc.scalar.activation(out=e0, in_=s2, func=AF.Exp, scale=temp_ap)
                em = attn.tile([C, C], f32r)
                nc.vector.tensor_mul(out=em, in0=e0, in1=bmasks)
                rs = attn.tile([C, 1], f32)
                nc.vector.reduce_sum(out=rs, in_=em, axis=AX.X)
                rinv = attn.tile([C, 1], f32)
                nc.vector.reciprocal(out=rinv, in_=rs)

                # L pairs: l8[:, p] = fp8(S_L * E diag(rinv) w1_pair_p).
                # Pair 4 slot 0 is tap8; its slot-1 "zero" k-tile carries the
                # fp8 quantization error of slot 0 (rhs window repeats).
                # w1s pair 4 slot 1 is wsc -> lsc8 with the same trick.
                rt = attn.tile([C, 5, 2, C], f32r)
                nc.vector.tensor_scalar_mul(out=rt, in0=w1s, scalar1=rinv)
                l8 = attn.tile([C, 5, 2, C], f8)
                lsc8 = attn.tile([C, 2, C], f8)
                for p in range(5):
                    lps = psW.tile([C, 2, C], f32, name=f"lps{p}", tag="lps")
                    nc.tensor.matmul(
                        out=lps, lhsT=em, rhs=rt[:, p], start=True, stop=True)
                    if p < 4:
                        nc.scalar.activation(
                            out=l8[:, p], in_=lps, func=AF.Copy, scale=S_L)
                    else:
                        nc.scalar.activation(
                            out=l8[:, 4, 0, :], in_=lps[:, 0, :], func=AF.Copy,
                            scale=S_L)
                        nc.vector.scalar_tensor_tensor(
                            out=l8[:, 4, 1, :], in0=lps[:, 0, :], scalar=S_L,
                            in1=l8[:, 4, 0, :], op0=ALU.mult, op1=ALU.subtract)
                        nc.scalar.activation(
                            out=lsc8[:, 0, :], in_=lps[:, 1, :], func=AF.Copy,
                            scale=S_L)
                        nc.vector.scalar_tensor_tensor(
                            out=lsc8[:, 1, :], in0=lps[:, 1, :], scalar=S_L,
                            in1=lsc8[:, 0, :], op0=ALU.mult, op1=ALU.subtract)

            # first x1 residual chunk before the band loop starts
            state = {}
            state["x1b"] = x1st.tile([C, 2048], bf16, tag="x1b", name="x1b0")
            nc.sync.dma_start(out=state["x1b"], in_=x1b16[:, 0:2048])

            # ============ phases C/D interleaved per 8-row band =============
            # C band b writes php rows 8b+1..8b+8; D band b's first matmul
            # half reads php rows <= 8b+5 (band b itself), its second half
            # rows <= 8b+9 (band b+1's first ACT half-pass).
            with (
                tc.tile_pool(name="psC", bufs=2, space="PSUM") as psC,
                tc.tile_pool(name="psD", bufs=2, space="PSUM") as psD,
                tc.tile_pool(name="psS2", bufs=1, space="PSUM") as psS2,
            ):
                def emit_C(b):
                    y0 = ROWS * b
                    halves = []
                    for h in range(2):
                        cps = psC.tile([C, 4, C], f32, name=f"cps{b}_{h}", tag="cps")
                        for i, (_, _, dy, dx, strd) in enumerate(PAIRS):
                            nc.tensor.matmul(
                                out=cps,
                                lhsT=l8[:, i],
                                rhs=pair_rhs(p2x, y0 + 4 * h + dy, dx, strd),
                                start=(i == 0), stop=(i == 4),
                                perf_mode=PM.DoubleRow,
                            )
                        halves.append(cps)
                    for h, cps in enumerate(halves):
                        r0 = 1 + y0 + 4 * h
                        nc.scalar.activation(
                            out=php[:, r0 : r0 + 4, 1 : 1 + C],
                            in_=cps, func=AF.Prelu, bias=b1_ap,
                            scale=S_PH / S_L, alpha=SLOPE,
                        )
                        # incremental reflect pad of the left/right columns
                        nc.gpsimd.tensor_copy(
                            out=php[:, r0 : r0 + 4, 0:1],
                            in_=php[:, r0 : r0 + 4, 2:3])
                        nc.gpsimd.tensor_copy(
                            out=php[:, r0 : r0 + 4, PW - 1 : PW],
                            in_=php[:, r0 : r0 + 4, PW - 3 : PW - 2])
                    if b == 0:
                        nc.gpsimd.tensor_copy(out=php[:, 0:1, :], in_=php[:, 2:3, :])
                    if b == NB - 1:
                        nc.gpsimd.tensor_copy(
                            out=php[:, H + 1 : H + 2, :], in_=php[:, H - 1 : H, :])

                def emit_D(b):
                    y0 = ROWS * b
                    if b % 2 == 0 and b + 2 < NB:  # prefetch next residual pair
                        nx = x1st.tile([C, 2048], bf16, tag="x1b")
                        nc.sync.dma_start(
                            out=nx, in_=x1b16[:, (b + 2) * 1024 : (b + 4) * 1024])
                        state["x1b_next"] = nx
                    x1b = state["x1b"][:, (b % 2) * 1024 : (b % 2 + 1) * 1024]
                    x1b = x1b.rearrange("p (a b) -> p a b", a=ROWS)
                    # order: dps h0 (band-b php rows only), shortcut, dps h1
                    # (needs band b+1's first ACT half-pass) last.
                    dps = psD.tile([C, ROWS, C], f32)
                    for h in range(2):
                        if h == 1:
                            sps = psS2.tile([C, ROWS, C], f32)
                            for g in range(2):
                                nc.tensor.matmul(
                                    out=sps[:, 4 * g : 4 * g + 4, :],
                                    lhsT=lsc8,
                                    rhs=pair_rhs(p2x, y0 + 4 * g + 1, 1, 0),
                                    start=True, stop=True,
                                    perf_mode=PM.DoubleRow,
                                )
                        for i, (_, _, dy, dx, strd) in enumerate(PAIRS):
                            nc.tensor.matmul(
                                out=dps[:, 4 * h : 4 * h + 4, :],
                                lhsT=w2s[:, i],
                                rhs=pair_rhs(php, y0 + 4 * h + dy, dx, strd),
                                start=(i == 0), stop=(i == 4),
                                perf_mode=PM.DoubleRow,
                            )
                    # epilogue: out = h3 + sps/S_L + (x1 + sc_b)  [sc_b folded
                    # into x1b on the host].  The last band runs it in 4-row
                    # halves so the closing ACT->DVE->DMA chain is short.
                    otile = ost.tile([C, 1024], bf16)
                    parts = (
                        [(0, ROWS)] if b < NB - 1 else [(0, 4), (4, 4)]
                    )
                    for r0, nr in parts:
                        h3 = bands.tile([C, nr, C], bf16, name=f"h3_{b}_{r0}",
                                        tag="h3")
                        nc.scalar.activation(
                            out=h3, in_=dps[:, r0 : r0 + nr, :], func=AF.Prelu,
                            bias=b2_ap, scale=1.0 / (S_2 * S_PH), alpha=SLOPE,
                        )
                        t = bands.tile([C, nr, C], bf16, name=f"t_{b}_{r0}",
                                       tag="t")
                        nc.vector.scalar_tensor_tensor(
                            out=t, in0=sps[:, r0 : r0 + nr, :], scalar=1.0 / S_L,
                            in1=x1b[:, r0 : r0 + nr, :],
                            op0=ALU.mult, op1=ALU.add)
                        ob = otile[:, r0 * C : (r0 + nr) * C]
                        ob = ob.rearrange("p (a b) -> p a b", a=nr)
                        nc.vector.tensor_add(out=ob, in0=t, in1=h3)
                        nc.sync.dma_start(
                            out=out[:, b * 1024 + r0 * C : b * 1024 + (r0 + nr) * C],
                            in_=otile[:, r0 * C : (r0 + nr) * C])
                    if b % 2 == 1 and "x1b_next" in state:
                        state["x1b"] = state.pop("x1b_next")

                for k in range(NB + 1):
                    if k < NB:
                        emit_C(k)
                    if k >= 1:
                        emit_D(k - 1)

    nc.compile()
    return nc


def _prep_consts(temperature, conv1_w, conv1_b, conv2_w, conv2_b,
                 dw_w, dw_b, sc_w, sc_b):
    f32 = np.float32
    f8 = ml_dtypes.float8_e4m3
    conv1_w = np.asarray(conv1_w, f32)
    conv2_w = np.asarray(conv2_w, f32)
    dw_w = np.asarray(dw_w, f32)
    sc_w = np.asarray(sc_w, f32)

    # conv1 taps as lhsT: wc1[ci, t, co] = conv1_w[co, ci, dy, dx], arranged
    # in DoubleRow pair order; pair-4 slot 1 is the shortcut weight.
    wc1 = conv1_w.transpose(1, 2, 3, 0).reshape(C, 9, C)
    w1p = np.empty((C, 5, 2, C), f32)
    for i, (ta, tb, _, _, _) in enumerate(PAIRS[:4]):
        w1p[:, i, 0] = wc1[:, ta]
        w1p[:, i, 1] = wc1[:, tb]
    w1p[:, 4, 0] = wc1[:, 8]
    w1p[:, 4, 1] = sc_w[:, :, 0, 0].T

    # fused conv2+dw taps (scaled to fp8 range), same pair order; pair-4
    # slot 1 carries the fp8 quantization error of tap 8.
    A2 = conv2_w[:, :, 0, 0]                      # [co, ci]
    Dw = dw_w[:, 0, :, :].reshape(C, 9)           # [co, t]
    wc2 = np.einsum("oc,ot->tco", A2, Dw) * S_2   # [t, ci, co]
    w2p = np.empty((C, 5, 2, C), f8)
    for i, (ta, tb, _, _, _) in enumerate(PAIRS[:4]):
        w2p[:, i, 0] = wc2[ta].astype(f8)
        w2p[:, i, 1] = wc2[tb].astype(f8)
    t8q = wc2[8].astype(f8)
    w2p[:, 4, 0] = t8q
    w2p[:, 4, 1] = (wc2[8] - t8q.astype(f32)).astype(f8)

    b2p = np.asarray(dw_b, f32) + np.asarray(conv2_b, f32) * Dw.sum(axis=1)
    temp_b = np.repeat(np.asarray(temperature, f32).reshape(HEADS), HEAD_C)
    sclv = np.stack(
        [np.asarray(conv1_b, f32) * S_PH, b2p, temp_b], axis=1)
    bmaskv = np.kron(np.eye(HEADS, dtype=f32), np.ones((HEAD_C, HEAD_C), f32))
    cpack = np.concatenate([sclv, bmaskv, np.eye(C, dtype=f32)], axis=1)
    return dict(
        w1p=np.ascontiguousarray(w1p), w2p=np.ascontiguousarray(w2p),
        cpack=np.ascontiguousarray(cpack),
    )


def kernel(
    x1, x2, temperature, conv1_w, conv1_b, conv2_w, conv2_b, dw_w, dw_b, sc_w, sc_b
):
    from concourse.bass_utils import run_bass_kernel_spmd

    if "nc" not in _cache:
        _cache["nc"] = _build_program()
    nc = _cache["nc"]

    f8 = ml_dtypes.float8_e4m3
    bf16 = ml_dtypes.bfloat16
    x1 = np.ascontiguousarray(np.asarray(x1, np.float32))
    x2 = np.ascontiguousarray(np.asarray(x2, np.float32))
    consts = _prep_consts(
        temperature, conv1_w, conv1_b, conv2_w, conv2_b, dw_w, dw_b, sc_w, sc_b)
    scb = np.asarray(sc_b, np.float32)[:, None]

    in_maps = []
    for b in range(B):
        x1f = x1[b].reshape(C, HW)
        # pretransposed fp8 x1 in DoubleRow chunk order [p, g, j, c] with
        # spatial index = 256g + 128j + p
        x1t = x1f.T.reshape(64, 2, 128, C).transpose(2, 0, 1, 3).reshape(C, HW)
        x2p = np.zeros((C, H + 2, PW), np.float32)
        x2p[:, 1 : H + 1, 1 : W + 1] = x2[b].reshape(C, H, W)
        m = dict(consts)
        m["x1t8"] = np.ascontiguousarray(x1t.astype(f8))
        m["x2p8"] = np.ascontiguousarray(x2p.astype(f8).reshape(C, (H + 2) * PW))
        m["x1b16"] = np.ascontiguousarray((x1f + scb).astype(bf16))
        in_maps.append(m)

    res = run_bass_kernel_spmd(nc, in_maps, core_ids=list(range(B)))
    outs = [
        res.results[b]["out"].astype(np.float32).reshape(C, H, W)
        for b in range(B)
    ]
    return np.stack(outs, axis=0)
